# revision 23
# baseline (speedup 1.0000x reference)
"""Bahdanau attention kernel for Trainium2 (8 NeuronCores, SPMD data-parallel).

Reference computation (per batch b):
    f_proj = features[b] @ W1_w + W1_b            # [T, U]
    h_proj = hidden[b] @ W2_w + W2_b              # [U]
    score  = tanh(f_proj + h_proj) @ V_w + V_b    # [T]
    attn   = softmax(score)                       # [T]
    context[b] = sum_t attn[t] * features[b, t]   # [D]

Sharding: data-parallel over batch (64 batches / 8 cores = 8 per core),
weights replicated.

Per-core dataflow (everything fp32; matmuls optionally in float32r):
  - F tiles [128(t), 512(d)] are DMA'd in natively, PE-transposed
    (via identity matmul) into F^T [128(d), t] for the main matmul.
  - main matmul computes f_proj TRANSPOSED: [u(part), t(free)] =
    W1_chunk^T @ F^T, so the (W1_b + h_proj) bias is a per-partition
    scalar that fuses into the ACT Tanh instruction.
  - score^T [1, t] = V^T @ tanh via M=1 matmuls; ACT Exp with fused
    accum_out produces both e = exp(score + V_b) and its running sum.
    No max-subtraction: |score| <= ||V||_1 + |V_b| ~ 18, safe in fp32.
  - e rows are transposed to columns with tiny 1x1-identity matmuls;
    context accumulates as e_col^T @ F_native; final scale by 1/sum(e).
"""

import sys

for _p in ("/opt/trn_rl_repo", "/opt/pypackages"):
    if _p not in sys.path:
        sys.path.insert(0, _p)

import numpy as np

B, T, D, U = 64, 2048, 512, 512
NCORES = 8
BPC = B // NCORES          # batches per core
PART = 128
DC = D // PART             # 4 contraction chunks
UC = U // PART             # 4 u chunks
TCHUNK = 512               # t columns processed per main-matmul group
TILES_PER_CHUNK = TCHUNK // PART          # 4
NCHUNKS = (BPC * T) // TCHUNK             # 32
CHUNKS_PER_BATCH = T // TCHUNK            # 4

MM_DT_NAME = "float32r"    # dtype tag for matmul operands


_BUILD_CACHE = {}


def build_bass(mm_dt_name=MM_DT_NAME):
    """Build + compile the per-core Bass program (same on all cores)."""
    if mm_dt_name in _BUILD_CACHE:
        return _BUILD_CACHE[mm_dt_name]

    import concourse.mybir as mybir
    import concourse.tile as tile
    from concourse import bacc
    from concourse.bass import ts
    from concourse.masks import make_identity

    f32 = mybir.dt.float32
    mdt = getattr(mybir.dt, mm_dt_name)
    ACT = mybir.ActivationFunctionType
    AX = mybir.AxisListType

    nc = bacc.Bacc("TRN2", target_bir_lowering=False, debug=False)

    feat = nc.dram_tensor("features", [BPC, T, D], mdt, kind="ExternalInput")
    hid = nc.dram_tensor("hidden", [BPC, D], mdt, kind="ExternalInput")
    w1 = nc.dram_tensor("W1_w", [D, U], mdt, kind="ExternalInput")
    b1 = nc.dram_tensor("W1_b", [U], f32, kind="ExternalInput")
    w2 = nc.dram_tensor("W2_w", [D, U], mdt, kind="ExternalInput")
    b2 = nc.dram_tensor("W2_b", [U], f32, kind="ExternalInput")
    vw = nc.dram_tensor("V_w", [U, 1], mdt, kind="ExternalInput")
    vb = nc.dram_tensor("V_b", [1], f32, kind="ExternalInput")
    out = nc.dram_tensor("context", [BPC, D], f32, kind="ExternalOutput")

    with tile.TileContext(nc) as tc:
        with (
            tc.tile_pool(name="consts", bufs=1) as consts,
            tc.tile_pool(name="fpool", bufs=16) as fpool,
            tc.tile_pool(name="ftb", bufs=3) as ftb,
            tc.tile_pool(name="tanh", bufs=3) as tanhp,
            tc.tile_pool(name="small", bufs=3) as small,
            tc.tile_pool(name="outp", bufs=2) as outp,
            tc.tile_pool(name="ps_mm", bufs=3, space="PSUM") as ps_mm,
            tc.tile_pool(name="ps_t", bufs=3, space="PSUM") as ps_t,
            tc.tile_pool(name="ps_s", bufs=1, space="PSUM") as ps_s,
            tc.tile_pool(name="ps_c", bufs=1, space="PSUM") as ps_c,
        ):
            # ---------------- constants / setup ----------------
            ident_f32 = consts.tile([PART, PART], f32)
            make_identity(nc, ident_f32)
            ident = consts.tile([PART, PART], mdt)
            nc.vector.tensor_copy(ident, ident_f32)

            hid_sb = consts.tile([BPC, D], mdt)
            nc.sync.dma_start(out=hid_sb, in_=hid.ap())

            # preload the first two chunks' F tiles so the PE can start on
            # their transposes before the (large) weight DMAs complete
            preloaded = {}
            for pch in (0, 1):
                pb = pch // CHUNKS_PER_BATCH
                pt0 = (pch % CHUNKS_PER_BATCH) * TCHUNK
                tiles = []
                for j in range(TILES_PER_CHUNK):
                    f_pre = fpool.tile([PART, D], mdt, tag="F", name=f"f_pre_{pch}_{j}")
                    nc.sync.dma_start(
                        out=f_pre,
                        in_=feat.ap()[pb, pt0 + j * PART : pt0 + (j + 1) * PART, :],
                    )
                    tiles.append(f_pre)
                preloaded[pch] = tiles

            w2_sb = consts.tile([PART, DC, U], mdt)
            nc.sync.dma_start(out=w2_sb, in_=w2.ap().rearrange("(c p) u -> p c u", p=PART))
            w1_sb = consts.tile([PART, DC, U], mdt)
            nc.sync.dma_start(out=w1_sb, in_=w1.ap().rearrange("(c p) u -> p c u", p=PART))
            v_sb = consts.tile([PART, UC], mdt)
            nc.sync.dma_start(out=v_sb, in_=vw.ap().rearrange("(c p) one -> p (c one)", p=PART))
            vb_sb = consts.tile([1, 1], f32)
            nc.sync.dma_start(out=vb_sb, in_=vb.ap().rearrange("(one x) -> one x", one=1))

            # W1_b + W2_b as per-partition columns [128, uc]
            b1_sb = consts.tile([PART, UC], f32)
            nc.sync.dma_start(out=b1_sb, in_=b1.ap().rearrange("(c p) -> p c", p=PART))
            b2_sb = consts.tile([PART, UC], f32)
            nc.sync.dma_start(out=b2_sb, in_=b2.ap().rearrange("(c p) -> p c", p=PART))
            b12_sb = consts.tile([PART, UC], f32)
            nc.vector.tensor_add(b12_sb, b1_sb, b2_sb)

            # hidden [BPC, D] -> hiddenT [128(d), dc, BPC]
            hidT_sb = consts.tile([PART, DC, BPC], mdt)
            bias_cols = consts.tile([PART, UC, BPC], f32)

            def emit_setup():
                # emitted after chunk 0's transposes so the PE isn't blocked
                # on the weight/hidden DMAs at kernel start
                for dc in range(DC):
                    ps_h = ps_t.tile([PART, TCHUNK], mdt, tag="T", name="ps_h")
                    nc.tensor.transpose(ps_h[:, 0:BPC], hid_sb[:, ts(dc, PART)], ident[0:BPC, 0:BPC])
                    nc.vector.tensor_copy(hidT_sb[:, dc, :], ps_h[:, 0:BPC])
                # h_projT[u, b] = sum_dc W2[dc]^T @ hiddenT[dc]  (+W2_b+W1_b)
                for uc in range(UC):
                    ps_h = ps_t.tile([PART, TCHUNK], f32, tag="T", name="ps_h2")
                    for dc in range(DC):
                        nc.tensor.matmul(
                            ps_h[:, 0:BPC],
                            w2_sb[:, dc, ts(uc, PART)],
                            hidT_sb[:, dc, :],
                            start=(dc == 0),
                            stop=(dc == DC - 1),
                        )
                    nc.vector.tensor_scalar_add(
                        bias_cols[:, uc, :], ps_h[:, 0:BPC], b12_sb[:, uc : uc + 1]
                    )

            # ---------------- main loop (epilogue deferred one chunk) ----------------
            prev = None          # chunk state awaiting its score/context stage
            batch_state = {}     # per-batch psum_ctx / running-sum tiles

            def emit_scores(st):
                b, cib = st["b"], st["cib"]
                if cib == 0:
                    batch_state["ps_ctx"] = ps_c.tile([1, D], f32, tag="ctx", name="ps_ctx")
                    batch_state["s_sb"] = small.tile([1, CHUNKS_PER_BATCH], f32, tag="ssum", name="s_sb")
                s_sb = batch_state["s_sb"]

                # score^T [1, t] = V^T @ tanh
                ps_sc = ps_s.tile([1, TCHUNK], f32, tag="score")
                for uc in range(UC):
                    nc.tensor.matmul(
                        ps_sc,
                        v_sb[:, uc : uc + 1],
                        st["tanh"][:, uc, :],
                        start=(uc == 0),
                        stop=(uc == UC - 1),
                    )
                # e = exp(score + V_b); row sum via explicit DVE reduction
                # (avoids relying on the split ACTIVATION_READ_ACCUMULATOR op)
                e_sb = small.tile([1, TCHUNK], mdt, tag="e_sb")
                nc.scalar.activation(e_sb, ps_sc, ACT.Exp, bias=vb_sb)
                nc.vector.reduce_sum(s_sb[:, cib : cib + 1], e_sb, axis=AX.X)
                st["e_sb"] = e_sb

            def emit_context(st):
                b, cib = st["b"], st["cib"]
                ps_ctx = batch_state["ps_ctx"]
                s_sb = batch_state["s_sb"]
                e_sb = st["e_sb"]
                # transpose e row -> columns via [1,0]-identity-row matmuls
                # (pairs of output columns keep fp32r ISA patterns even)
                ps_e = ps_t.tile([PART, 2 * TILES_PER_CHUNK], f32, tag="T")
                for j in range(TILES_PER_CHUNK):
                    nc.tensor.matmul(
                        ps_e[:, 2 * j : 2 * j + 2],
                        e_sb[0:1, ts(j, PART)],
                        ident[0:1, 0:2],
                        start=True,
                        stop=True,
                    )
                e_col = small.tile([PART, TILES_PER_CHUNK], mdt, tag="e_col")
                nc.vector.tensor_copy(
                    e_col, ps_e.rearrange("p (j two) -> p two j", two=2)[:, 0, :]
                )
                # context accumulation: ps_ctx [1, D] += e_col_j^T @ F_j
                for j in range(TILES_PER_CHUNK):
                    nc.tensor.matmul(
                        ps_ctx,
                        e_col[:, j : j + 1],
                        st["f_tiles"][j],
                        start=(cib == 0 and j == 0),
                        stop=(cib == CHUNKS_PER_BATCH - 1 and j == TILES_PER_CHUNK - 1),
                    )
                if cib == CHUNKS_PER_BATCH - 1:
                    ssum = small.tile([1, 1], f32, tag="ssum1")
                    nc.vector.reduce_sum(ssum, s_sb, axis=AX.X)
                    rec = small.tile([1, 1], f32, tag="rec")
                    nc.vector.reciprocal(rec, ssum)
                    ctx_sb = outp.tile([1, D], f32, tag="ctx_sb")
                    nc.vector.tensor_scalar_mul(ctx_sb, ps_ctx, rec)
                    nc.sync.dma_start(out=out.ap()[b : b + 1, :], in_=ctx_sb)

            s1_state = {}

            def emit_s1(c):
                sb_ = c // CHUNKS_PER_BATCH
                st0 = (c % CHUNKS_PER_BATCH) * TCHUNK
                f_tiles = []
                ftile_big = ftb.tile([PART, DC, TCHUNK], mdt, tag="FT", name="ftile_big")
                for j in range(TILES_PER_CHUNK):
                    if c in preloaded:
                        f_ij = preloaded[c][j]
                    else:
                        f_ij = fpool.tile([PART, D], mdt, tag="F", name="f_ij")
                        nc.sync.dma_start(
                            out=f_ij,
                            in_=feat.ap()[sb_, st0 + j * PART : st0 + (j + 1) * PART, :],
                        )
                    f_tiles.append(f_ij)
                    ps_tr = ps_t.tile([PART, TCHUNK], mdt, tag="T", name="ps_tr")
                    for dc in range(DC):
                        nc.tensor.transpose(
                            ps_tr[:, ts(dc, PART)], f_ij[:, ts(dc, PART)], ident
                        )
                    nc.vector.tensor_copy(
                        ftile_big[:, :, ts(j, PART)],
                        ps_tr.rearrange("p (c t) -> p c t", c=DC),
                    )
                s1_state[c] = (f_tiles, ftile_big)

            emit_s1(0)

            for chunk in range(NCHUNKS + 1):
                # V-dot + exp of the previous chunk lead this chunk
                if prev is not None:
                    emit_scores(prev)

                # transpose stage one chunk ahead, so this chunk's mains
                # never wait on the F^T copies
                if chunk + 1 < NCHUNKS:
                    emit_s1(chunk + 1)

                if chunk == 0:
                    emit_setup()

                if chunk < NCHUNKS:
                    b = chunk // CHUNKS_PER_BATCH
                    cib = chunk % CHUNKS_PER_BATCH
                    f_tiles, ftile_big = s1_state.pop(chunk)

                # context stage of the PREVIOUS chunk overlaps this chunk's mains
                if prev is not None:
                    emit_context(prev)
                    prev = None

                if chunk < NCHUNKS:
                    # S2: main matmul + tanh (transposed layout [u, t])
                    tanh_sb = tanhp.tile([PART, UC, TCHUNK], mdt, tag="tanh")
                    for uc in range(UC):
                        ps_f = ps_mm.tile([PART, TCHUNK], f32, tag="mm")
                        for dc in range(DC):
                            nc.tensor.matmul(
                                ps_f,
                                w1_sb[:, dc, ts(uc, PART)],
                                ftile_big[:, dc, :],
                                start=(dc == 0),
                                stop=(dc == DC - 1),
                            )
                        nc.scalar.activation(
                            tanh_sb[:, uc, :],
                            ps_f,
                            ACT.Tanh,
                            bias=bias_cols[:, uc, b : b + 1],
                        )
                    prev = {"b": b, "cib": cib, "tanh": tanh_sb, "f_tiles": f_tiles}

    nc.compile()
    _BUILD_CACHE[mm_dt_name] = nc
    return nc


def kernel(**inputs):
    from concourse.bass_utils import run_bass_kernel_spmd

    nc = build_bass()

    feat = np.ascontiguousarray(np.asarray(inputs["features"], dtype=np.float32))
    hid = np.ascontiguousarray(np.asarray(inputs["hidden"], dtype=np.float32))
    shared = {
        k: np.ascontiguousarray(np.asarray(inputs[k], dtype=np.float32))
        for k in ("W1_w", "W1_b", "W2_w", "W2_b", "V_w", "V_b")
    }
    in_maps = []
    for c in range(NCORES):
        m = dict(shared)
        m["features"] = feat[c * BPC : (c + 1) * BPC]
        m["hidden"] = hid[c * BPC : (c + 1) * BPC]
        in_maps.append(m)

    res = run_bass_kernel_spmd(nc, in_maps, list(range(NCORES)))
    return np.concatenate([res.results[c]["context"] for c in range(NCORES)], axis=0)


# revision 24
# speedup vs baseline: 1.0055x; 1.0055x over previous
"""Bahdanau attention kernel for Trainium2 (8 NeuronCores, SPMD data-parallel).

Reference computation (per batch b):
    f_proj = features[b] @ W1_w + W1_b            # [T, U]
    h_proj = hidden[b] @ W2_w + W2_b              # [U]
    score  = tanh(f_proj + h_proj) @ V_w + V_b    # [T]
    attn   = softmax(score)                       # [T]
    context[b] = sum_t attn[t] * features[b, t]   # [D]

Sharding: data-parallel over batch (64 batches / 8 cores = 8 per core),
weights replicated.

Per-core dataflow (everything fp32; matmuls optionally in float32r):
  - F tiles [128(t), 512(d)] are DMA'd in natively, PE-transposed
    (via identity matmul) into F^T [128(d), t] for the main matmul.
  - main matmul computes f_proj TRANSPOSED: [u(part), t(free)] =
    W1_chunk^T @ F^T, so the (W1_b + h_proj) bias is a per-partition
    scalar that fuses into the ACT Tanh instruction.
  - score^T [1, t] = V^T @ tanh via M=1 matmuls; ACT Exp with fused
    accum_out produces both e = exp(score + V_b) and its running sum.
    No max-subtraction: |score| <= ||V||_1 + |V_b| ~ 18, safe in fp32.
  - e rows are transposed to columns with tiny 1x1-identity matmuls;
    context accumulates as e_col^T @ F_native; final scale by 1/sum(e).
"""

import sys

for _p in ("/opt/trn_rl_repo", "/opt/pypackages"):
    if _p not in sys.path:
        sys.path.insert(0, _p)

import numpy as np

B, T, D, U = 64, 2048, 512, 512
NCORES = 8
BPC = B // NCORES          # batches per core
PART = 128
DC = D // PART             # 4 contraction chunks
UC = U // PART             # 4 u chunks
TCHUNK = 512               # t columns processed per main-matmul group
TILES_PER_CHUNK = TCHUNK // PART          # 4
NCHUNKS = (BPC * T) // TCHUNK             # 32
CHUNKS_PER_BATCH = T // TCHUNK            # 4

MM_DT_NAME = "float32r"    # dtype tag for matmul operands


_BUILD_CACHE = {}


def build_bass(mm_dt_name=MM_DT_NAME):
    """Build + compile the per-core Bass program (same on all cores)."""
    if mm_dt_name in _BUILD_CACHE:
        return _BUILD_CACHE[mm_dt_name]

    import concourse.mybir as mybir
    import concourse.tile as tile
    from concourse import bacc
    from concourse.bass import ts
    from concourse.masks import make_identity

    f32 = mybir.dt.float32
    mdt = getattr(mybir.dt, mm_dt_name)
    ACT = mybir.ActivationFunctionType
    AX = mybir.AxisListType

    nc = bacc.Bacc("TRN2", target_bir_lowering=False, debug=False)

    feat = nc.dram_tensor("features", [BPC, T, D], mdt, kind="ExternalInput")
    hid = nc.dram_tensor("hidden", [BPC, D], mdt, kind="ExternalInput")
    w1 = nc.dram_tensor("W1_w", [D, U], mdt, kind="ExternalInput")
    b1 = nc.dram_tensor("W1_b", [U], f32, kind="ExternalInput")
    w2 = nc.dram_tensor("W2_w", [D, U], mdt, kind="ExternalInput")
    b2 = nc.dram_tensor("W2_b", [U], f32, kind="ExternalInput")
    vw = nc.dram_tensor("V_w", [U, 1], mdt, kind="ExternalInput")
    vb = nc.dram_tensor("V_b", [1], f32, kind="ExternalInput")
    out = nc.dram_tensor("context", [BPC, D], f32, kind="ExternalOutput")

    with tile.TileContext(nc) as tc:
        with (
            tc.tile_pool(name="consts", bufs=1) as consts,
            tc.tile_pool(name="fpool", bufs=16) as fpool,
            tc.tile_pool(name="ftb", bufs=3) as ftb,
            tc.tile_pool(name="tanh", bufs=3) as tanhp,
            tc.tile_pool(name="small", bufs=3) as small,
            tc.tile_pool(name="outp", bufs=2) as outp,
            tc.tile_pool(name="ps_mm", bufs=3, space="PSUM") as ps_mm,
            tc.tile_pool(name="ps_t", bufs=3, space="PSUM") as ps_t,
            tc.tile_pool(name="ps_s", bufs=1, space="PSUM") as ps_s,
            tc.tile_pool(name="ps_c", bufs=1, space="PSUM") as ps_c,
        ):
            # ---------------- constants / setup ----------------
            ident_f32 = consts.tile([PART, PART], f32)
            make_identity(nc, ident_f32)
            ident = consts.tile([PART, PART], mdt)
            nc.vector.tensor_copy(ident, ident_f32)

            # preload the first two chunks' F tiles so the PE can start on
            # their transposes before the (large) weight DMAs complete
            preloaded = {}
            for pch in (0, 1):
                pb = pch // CHUNKS_PER_BATCH
                pt0 = (pch % CHUNKS_PER_BATCH) * TCHUNK
                tiles = []
                for j in range(TILES_PER_CHUNK):
                    f_pre = fpool.tile([PART, D], mdt, tag="F", name=f"f_pre_{pch}_{j}")
                    nc.sync.dma_start(
                        out=f_pre,
                        in_=feat.ap()[pb, pt0 + j * PART : pt0 + (j + 1) * PART, :],
                    )
                    tiles.append(f_pre)
                preloaded[pch] = tiles

            w1_sb = consts.tile([PART, DC, U], mdt)
            nc.sync.dma_start(out=w1_sb, in_=w1.ap().rearrange("(c p) u -> p c u", p=PART))
            w2_sb = consts.tile([PART, DC, U], mdt)
            nc.sync.dma_start(out=w2_sb, in_=w2.ap().rearrange("(c p) u -> p c u", p=PART))
            v_sb = consts.tile([PART, UC], mdt)
            nc.sync.dma_start(out=v_sb, in_=vw.ap().rearrange("(c p) one -> p (c one)", p=PART))
            vb_sb = consts.tile([1, 1], f32)
            nc.sync.dma_start(out=vb_sb, in_=vb.ap().rearrange("(one x) -> one x", one=1))

            # W1_b + W2_b as per-partition columns [128, uc]
            b1_sb = consts.tile([PART, UC], f32)
            nc.sync.dma_start(out=b1_sb, in_=b1.ap().rearrange("(c p) -> p c", p=PART))
            b2_sb = consts.tile([PART, UC], f32)
            nc.sync.dma_start(out=b2_sb, in_=b2.ap().rearrange("(c p) -> p c", p=PART))
            b12_sb = consts.tile([PART, UC], f32)
            nc.vector.tensor_add(b12_sb, b1_sb, b2_sb)

            # hidden [BPC, D] -> hiddenT [128(d), dc, BPC]
            hid_sb = consts.tile([BPC, D], mdt)
            nc.sync.dma_start(out=hid_sb, in_=hid.ap())
            hidT_sb = consts.tile([PART, DC, BPC], mdt)
            bias_cols = consts.tile([PART, UC, BPC], f32)

            def emit_setup():
                # emitted after chunk 0's transposes so the PE isn't blocked
                # on the weight/hidden DMAs at kernel start
                for dc in range(DC):
                    ps_h = ps_t.tile([PART, TCHUNK], mdt, tag="T", name="ps_h")
                    nc.tensor.transpose(ps_h[:, 0:BPC], hid_sb[:, ts(dc, PART)], ident[0:BPC, 0:BPC])
                    nc.vector.tensor_copy(hidT_sb[:, dc, :], ps_h[:, 0:BPC])
                # h_projT[u, b] = sum_dc W2[dc]^T @ hiddenT[dc]  (+W2_b+W1_b)
                for uc in range(UC):
                    ps_h = ps_t.tile([PART, TCHUNK], f32, tag="T", name="ps_h2")
                    for dc in range(DC):
                        nc.tensor.matmul(
                            ps_h[:, 0:BPC],
                            w2_sb[:, dc, ts(uc, PART)],
                            hidT_sb[:, dc, :],
                            start=(dc == 0),
                            stop=(dc == DC - 1),
                        )
                    nc.vector.tensor_scalar_add(
                        bias_cols[:, uc, :], ps_h[:, 0:BPC], b12_sb[:, uc : uc + 1]
                    )

            # ---------------- main loop (epilogue deferred one chunk) ----------------
            prev = None          # chunk state awaiting its score/context stage
            batch_state = {}     # per-batch psum_ctx / running-sum tiles

            def emit_scores(st):
                b, cib = st["b"], st["cib"]
                if cib == 0:
                    batch_state["ps_ctx"] = ps_c.tile([1, D], f32, tag="ctx", name="ps_ctx")
                    batch_state["s_sb"] = small.tile([1, CHUNKS_PER_BATCH], f32, tag="ssum", name="s_sb")
                s_sb = batch_state["s_sb"]

                # score^T [1, t] = V^T @ tanh
                ps_sc = ps_s.tile([1, TCHUNK], f32, tag="score")
                for uc in range(UC):
                    nc.tensor.matmul(
                        ps_sc,
                        v_sb[:, uc : uc + 1],
                        st["tanh"][:, uc, :],
                        start=(uc == 0),
                        stop=(uc == UC - 1),
                    )
                # e = exp(score + V_b); row sum via explicit DVE reduction
                # (avoids relying on the split ACTIVATION_READ_ACCUMULATOR op)
                e_sb = small.tile([1, TCHUNK], mdt, tag="e_sb")
                nc.scalar.activation(e_sb, ps_sc, ACT.Exp, bias=vb_sb)
                nc.vector.reduce_sum(s_sb[:, cib : cib + 1], e_sb, axis=AX.X)
                st["e_sb"] = e_sb

            def emit_context(st):
                b, cib = st["b"], st["cib"]
                ps_ctx = batch_state["ps_ctx"]
                s_sb = batch_state["s_sb"]
                e_sb = st["e_sb"]
                # transpose e row -> columns via [1,0]-identity-row matmuls
                # (pairs of output columns keep fp32r ISA patterns even)
                ps_e = ps_t.tile([PART, 2 * TILES_PER_CHUNK], f32, tag="T")
                for j in range(TILES_PER_CHUNK):
                    nc.tensor.matmul(
                        ps_e[:, 2 * j : 2 * j + 2],
                        e_sb[0:1, ts(j, PART)],
                        ident[0:1, 0:2],
                        start=True,
                        stop=True,
                    )
                e_col = small.tile([PART, TILES_PER_CHUNK], mdt, tag="e_col")
                nc.vector.tensor_copy(
                    e_col, ps_e.rearrange("p (j two) -> p two j", two=2)[:, 0, :]
                )
                # context accumulation: ps_ctx [1, D] += e_col_j^T @ F_j
                for j in range(TILES_PER_CHUNK):
                    nc.tensor.matmul(
                        ps_ctx,
                        e_col[:, j : j + 1],
                        st["f_tiles"][j],
                        start=(cib == 0 and j == 0),
                        stop=(cib == CHUNKS_PER_BATCH - 1 and j == TILES_PER_CHUNK - 1),
                    )
                if cib == CHUNKS_PER_BATCH - 1:
                    ssum = small.tile([1, 1], f32, tag="ssum1")
                    nc.vector.reduce_sum(ssum, s_sb, axis=AX.X)
                    rec = small.tile([1, 1], f32, tag="rec")
                    nc.vector.reciprocal(rec, ssum)
                    ctx_sb = outp.tile([1, D], f32, tag="ctx_sb")
                    nc.vector.tensor_scalar_mul(ctx_sb, ps_ctx, rec)
                    nc.sync.dma_start(out=out.ap()[b : b + 1, :], in_=ctx_sb)

            s1_state = {}

            def emit_s1(c):
                sb_ = c // CHUNKS_PER_BATCH
                st0 = (c % CHUNKS_PER_BATCH) * TCHUNK
                f_tiles = []
                ftile_big = ftb.tile([PART, DC, TCHUNK], mdt, tag="FT", name="ftile_big")
                for j in range(TILES_PER_CHUNK):
                    if c in preloaded:
                        f_ij = preloaded[c][j]
                    else:
                        f_ij = fpool.tile([PART, D], mdt, tag="F", name="f_ij")
                        nc.sync.dma_start(
                            out=f_ij,
                            in_=feat.ap()[sb_, st0 + j * PART : st0 + (j + 1) * PART, :],
                        )
                    f_tiles.append(f_ij)
                    ps_tr = ps_t.tile([PART, TCHUNK], mdt, tag="T", name="ps_tr")
                    for dc in range(DC):
                        nc.tensor.transpose(
                            ps_tr[:, ts(dc, PART)], f_ij[:, ts(dc, PART)], ident
                        )
                    nc.vector.tensor_copy(
                        ftile_big[:, :, ts(j, PART)],
                        ps_tr.rearrange("p (c t) -> p c t", c=DC),
                    )
                s1_state[c] = (f_tiles, ftile_big)

            emit_s1(0)

            for chunk in range(NCHUNKS + 1):
                # V-dot + exp of the previous chunk lead this chunk
                if prev is not None:
                    emit_scores(prev)

                # transpose stage one chunk ahead, so this chunk's mains
                # never wait on the F^T copies
                if chunk + 1 < NCHUNKS:
                    emit_s1(chunk + 1)

                if chunk == 0:
                    emit_setup()

                if chunk < NCHUNKS:
                    b = chunk // CHUNKS_PER_BATCH
                    cib = chunk % CHUNKS_PER_BATCH
                    f_tiles, ftile_big = s1_state.pop(chunk)

                # context stage of the PREVIOUS chunk overlaps this chunk's mains
                if prev is not None:
                    emit_context(prev)
                    prev = None

                if chunk < NCHUNKS:
                    # S2: main matmul + tanh (transposed layout [u, t])
                    tanh_sb = tanhp.tile([PART, UC, TCHUNK], mdt, tag="tanh")
                    for uc in range(UC):
                        ps_f = ps_mm.tile([PART, TCHUNK], f32, tag="mm")
                        for dc in range(DC):
                            nc.tensor.matmul(
                                ps_f,
                                w1_sb[:, dc, ts(uc, PART)],
                                ftile_big[:, dc, :],
                                start=(dc == 0),
                                stop=(dc == DC - 1),
                            )
                        nc.scalar.activation(
                            tanh_sb[:, uc, :],
                            ps_f,
                            ACT.Tanh,
                            bias=bias_cols[:, uc, b : b + 1],
                        )
                    prev = {"b": b, "cib": cib, "tanh": tanh_sb, "f_tiles": f_tiles}

    nc.compile()
    _BUILD_CACHE[mm_dt_name] = nc
    return nc


def kernel(**inputs):
    from concourse.bass_utils import run_bass_kernel_spmd

    nc = build_bass()

    feat = np.ascontiguousarray(np.asarray(inputs["features"], dtype=np.float32))
    hid = np.ascontiguousarray(np.asarray(inputs["hidden"], dtype=np.float32))
    shared = {
        k: np.ascontiguousarray(np.asarray(inputs[k], dtype=np.float32))
        for k in ("W1_w", "W1_b", "W2_w", "W2_b", "V_w", "V_b")
    }
    in_maps = []
    for c in range(NCORES):
        m = dict(shared)
        m["features"] = feat[c * BPC : (c + 1) * BPC]
        m["hidden"] = hid[c * BPC : (c + 1) * BPC]
        in_maps.append(m)

    res = run_bass_kernel_spmd(nc, in_maps, list(range(NCORES)))
    return np.concatenate([res.results[c]["context"] for c in range(NCORES)], axis=0)
